# revision 4
# baseline (speedup 1.0000x reference)
"""BitLinear (ternary-weight linear + global activation requant) on 8 TRN2 cores.

Computation (see reference):
    wq  = ternarize(weight * scale, thr = 0.7*mean|weight*scale|)   # {-1,0,+1}
    out = x @ wq.T + bias
    s   = 255 / (max(out) - min(out));  out = round(out*s)/s

Sharding: 2x4 grid over (tokens, out_features).  Each core computes a
[4096 tok, 1024 out] shard contracting over the full K=4096.
x is pre-transposed/cast to bf16 on the host (layout work only); the ternary
threshold and the activation max/min are reduced across cores on-device with
two tiny AllReduces, exactly as the reference math requires.
"""

import numpy as np
import ml_dtypes

import concourse.bass as bass
import concourse.mybir as mybir
import concourse.tile as tile
from concourse import bacc
from concourse import bass_utils

F32 = mybir.dt.float32
BF16 = mybir.dt.bfloat16

# Full problem shape
B, S, D_IN, D_OUT = 4, 2048, 4096, 4096
N_CORES = 8
GRID_R, GRID_C = 2, 4  # token shards x out-feature shards

# Round-to-nearest-even magic constant (valid for |y| < 2^22)
RND_C = float(np.float32(12582912.0))  # 1.5 * 2^23


def build_kernel(
    tok_per_core: int,
    k_dim: int,
    out_per_core: int,
    tok_block: int,
    n_weight_copies: int,
    debug: bool = False,
):
    """Build + compile the per-core SPMD Bass program.

    n_weight_copies: how many cores hold the same W shard (= GRID_R); the
    threshold AllReduce over all cores over-counts by this factor.
    """
    KO = k_dim // 128
    SUBS = tok_block // 128
    OGS = max(1, out_per_core // 512)
    OGW = min(512, out_per_core)  # o-group width
    N_BLOCKS = tok_per_core // tok_block
    assert KO * 128 == k_dim and SUBS * 128 == tok_block
    assert OGS * OGW == out_per_core and N_BLOCKS * tok_block == tok_per_core

    nc = bacc.Bacc(
        "TRN2",
        target_bir_lowering=False,
        debug=debug,
        enable_asserts=False,
        num_devices=N_CORES,
    )

    xt = nc.declare_dram_parameter("xt", [N_BLOCKS, k_dim, tok_block], BF16, isOutput=False)
    wt = nc.declare_dram_parameter("wt", [k_dim, out_per_core], F32, isOutput=False)
    biasv = nc.declare_dram_parameter("biasv", [out_per_core], F32, isOutput=False)
    scalev = nc.declare_dram_parameter("scalev", [1], F32, isOutput=False)
    out = nc.declare_dram_parameter("outv", [tok_per_core, out_per_core], F32, isOutput=True)

    # raw (pre-quant) output staging in DRAM
    stage = nc.dram_tensor("stage", [tok_per_core, out_per_core], F32)

    xt_ap = xt.ap()
    wt_ap = wt.ap()
    stage_ap = stage.ap()
    out_ap = out.ap()

    n_drains = N_BLOCKS * SUBS * OGS

    with tile.TileContext(nc) as tc:
        with (
            tc.tile_pool(name="const", bufs=1) as const_pool,
            tc.tile_pool(name="wslab", bufs=5) as wslab_pool,
            tc.tile_pool(name="wq", bufs=1) as wq_pool,
            tc.tile_pool(name="xbuf", bufs=2) as x_pool,
            tc.tile_pool(name="drain", bufs=3) as drain_pool,
            tc.tile_pool(name="qt", bufs=2) as q_pool,
            tc.tile_pool(name="psum", bufs=1, space="PSUM") as psum_pool,
            tc.tile_pool(name="dram", bufs=1, space="DRAM") as dram_pool,
        ):
            # ---- tiny constants --------------------------------------------
            scale_sb = const_pool.tile([1, 1], F32, tag="scale_sb")
            nc.sync.dma_start(scale_sb, scalev.ap()[None, :])
            scale_b = const_pool.tile([128, 1], F32, tag="scale_b")
            nc.gpsimd.partition_broadcast(scale_b, scale_sb)

            bias_sb = const_pool.tile([1, out_per_core], F32, tag="bias_sb")
            nc.sync.dma_start(bias_sb, biasv.ap()[None, :])
            bias_b = const_pool.tile([128, out_per_core], F32, tag="bias_b")
            nc.gpsimd.partition_broadcast(bias_b, bias_sb)

            # ---- phase 1: |W*scale| sum + AllReduce -> ternary threshold ---
            wsum = const_pool.tile([128, KO], F32, tag="wsum")
            for ko in range(KO):
                wslab = wslab_pool.tile([128, out_per_core], F32, tag="wslab")
                nc.sync.dma_start(wslab, wt_ap[ko * 128:(ko + 1) * 128, :])
                ws = wslab_pool.tile([128, out_per_core], F32, tag="wslab")
                nc.vector.tensor_scalar_mul(ws, wslab, scale_b)
                nc.vector.tensor_reduce(
                    wsum[:, ko:ko + 1], ws, axis=mybir.AxisListType.X,
                    op=mybir.AluOpType.add, apply_absolute_value=True,
                )

            wsum1 = const_pool.tile([128, 1], F32, tag="wsum1")
            nc.vector.tensor_reduce(
                wsum1, wsum, axis=mybir.AxisListType.X, op=mybir.AluOpType.add
            )
            wsum_all = const_pool.tile([128, 1], F32, tag="wsum_all")
            nc.gpsimd.partition_all_reduce(
                wsum_all, wsum1, 128, bass.bass_isa.ReduceOp.add
            )

            cc1_in = dram_pool.tile([1, 1], F32)
            cc1_out = dram_pool.tile([1, 1], F32)
            nc.sync.dma_start(cc1_in, wsum_all[0:1, :])
            nc.gpsimd.collective_compute(
                "AllReduce",
                mybir.AluOpType.add,
                replica_groups=[list(range(N_CORES))],
                ins=[cc1_in.opt()],
                outs=[cc1_out.opt()],
            )
            s_glob = const_pool.tile([1, 1], F32, tag="s_glob")
            nc.sync.dma_start(s_glob, cc1_out)

            # thr2 = [t, -t];  t = 0.7 * (S_global / n_copies) / (k_dim*D_OUT)
            n_w_elems = float(k_dim * GRID_C * out_per_core)
            tcoef = float(np.float32(0.7) / np.float64(n_weight_copies * n_w_elems))
            thr_c = const_pool.tile([1, 2], F32, tag="thr_c")
            nc.vector.memset(thr_c[:, 0:1], tcoef)
            nc.vector.memset(thr_c[:, 1:2], -tcoef)
            thr2 = const_pool.tile([1, 2], F32, tag="thr2")
            nc.vector.tensor_scalar_mul(thr2, thr_c, s_glob)
            thr_b = const_pool.tile([128, 2], F32, tag="thr_b")
            nc.gpsimd.partition_broadcast(thr_b, thr2)

            # ---- phase 1b: ternarize W -> resident bf16 wq [128, KO, out] --
            wq = wq_pool.tile([128, KO, out_per_core], BF16, tag="wq")
            for ko in range(KO):
                wslab = wslab_pool.tile([128, out_per_core], F32, tag="wslab")
                nc.sync.dma_start(wslab, wt_ap[ko * 128:(ko + 1) * 128, :])
                ws = wslab_pool.tile([128, out_per_core], F32, tag="wslab")
                nc.vector.tensor_scalar_mul(ws, wslab, scale_b)
                g = wslab_pool.tile([128, out_per_core], BF16, tag="tern_g")
                l = wslab_pool.tile([128, out_per_core], BF16, tag="tern_l")
                nc.vector.tensor_scalar(
                    g, ws, thr_b[:, 0:1], None, mybir.AluOpType.is_gt
                )
                nc.vector.tensor_scalar(
                    l, ws, thr_b[:, 1:2], None, mybir.AluOpType.is_lt
                )
                nc.vector.tensor_sub(wq[:, ko, :], g, l)

            # ---- phase 2: matmul blocks ------------------------------------
            maxst = const_pool.tile([128, n_drains], F32, tag="maxst")
            minst = const_pool.tile([128, n_drains], F32, tag="minst")

            for blk in range(N_BLOCKS):
                x_tile = x_pool.tile([128, KO, tok_block], BF16, tag="x_tile")
                nc.sync.dma_start(
                    x_tile, xt_ap[blk].rearrange("(ko p) t -> p ko t", p=128)
                )
                psums = [
                    [
                        psum_pool.tile([128, OGW], F32, name=f"ps_{sub}_{og}")
                        for og in range(OGS)
                    ]
                    for sub in range(SUBS)
                ]
                for ko in range(KO):
                    for sub in range(SUBS):
                        lhsT = x_tile[:, ko, sub * 128:(sub + 1) * 128]
                        for og in range(OGS):
                            nc.tensor.matmul(
                                psums[sub][og],
                                lhsT,
                                wq[:, ko, og * OGW:(og + 1) * OGW],
                                start=(ko == 0),
                                stop=(ko == KO - 1),
                            )
                # drain: +bias, track max/min, stage raw fp32 to DRAM
                for sub in range(SUBS):
                    tok0 = blk * tok_block + sub * 128
                    for og in range(OGS):
                        d = drain_pool.tile([128, OGW], F32, tag="drain")
                        nc.vector.tensor_add(
                            d, psums[sub][og], bias_b[:, og * OGW:(og + 1) * OGW]
                        )
                        idx = (blk * SUBS + sub) * OGS + og
                        nc.vector.tensor_reduce(
                            maxst[:, idx:idx + 1], d, axis=mybir.AxisListType.X,
                            op=mybir.AluOpType.max,
                        )
                        nc.vector.tensor_reduce(
                            minst[:, idx:idx + 1], d, axis=mybir.AxisListType.X,
                            op=mybir.AluOpType.min,
                        )
                        nc.sync.dma_start(
                            stage_ap[tok0:tok0 + 128, og * OGW:(og + 1) * OGW], d
                        )

            # ---- phase 3: global max/min -> s -> requantize ----------------
            lmax = const_pool.tile([128, 1], F32, tag="lmax")
            lmin = const_pool.tile([128, 1], F32, tag="lmin")
            nc.vector.tensor_reduce(
                lmax, maxst, axis=mybir.AxisListType.X, op=mybir.AluOpType.max
            )
            nc.vector.tensor_reduce(
                lmin, minst, axis=mybir.AxisListType.X, op=mybir.AluOpType.min
            )
            st2 = const_pool.tile([128, 2], F32, tag="st2")
            nc.vector.tensor_copy(out=st2[:, 0:1], in_=lmax)
            nc.vector.tensor_scalar_mul(st2[:, 1:2], lmin, -1.0)
            st2r = const_pool.tile([128, 2], F32, tag="st2r")
            nc.gpsimd.partition_all_reduce(
                st2r, st2, 128, bass.bass_isa.ReduceOp.max
            )

            cc2_in = dram_pool.tile([1, 2], F32)
            cc2_out = dram_pool.tile([1, 2], F32)
            nc.sync.dma_start(cc2_in, st2r[0:1, :])
            nc.gpsimd.collective_compute(
                "AllReduce",
                mybir.AluOpType.max,
                replica_groups=[list(range(N_CORES))],
                ins=[cc2_in.opt()],
                outs=[cc2_out.opt()],
            )
            gst = const_pool.tile([1, 2], F32, tag="gst")
            nc.sync.dma_start(gst, cc2_out)

            rng = const_pool.tile([1, 1], F32, tag="rng")  # max - min
            nc.vector.tensor_reduce(
                rng, gst, axis=mybir.AxisListType.X, op=mybir.AluOpType.add
            )

            def accurate_recip(out_ap, in_ap, tag):
                # r1 = r0*(2 - x*r0), one Newton step on InstReciprocal
                r0 = const_pool.tile([1, 1], F32, tag=f"{tag}_r0")
                nc.vector.reciprocal(r0, in_ap)
                e = const_pool.tile([1, 1], F32, tag=f"{tag}_e")
                nc.vector.tensor_scalar(
                    e, in_ap, r0, None, mybir.AluOpType.mult
                )
                nc.vector.tensor_scalar(
                    e, e, -1.0, 2.0, mybir.AluOpType.mult, mybir.AluOpType.add
                )
                nc.vector.tensor_mul(out_ap, r0, e)

            sq = const_pool.tile([1, 2], F32, tag="sq")  # [s, 1/s]
            rinv = const_pool.tile([1, 1], F32, tag="rinv")
            accurate_recip(rinv, rng, "rr")
            nc.vector.tensor_scalar_mul(sq[:, 0:1], rinv, 255.0)
            accurate_recip(sq[:, 1:2], sq[:, 0:1], "si")
            sq_b = const_pool.tile([128, 2], F32, tag="sq_b")
            nc.gpsimd.partition_broadcast(sq_b, sq)

            # requantize: q = (round(y*s))/s with RNE via +/- 1.5*2^23
            CHUNK = 2  # 128-row groups per quantize tile
            n_chunks = (tok_per_core // 128) // CHUNK
            stage_r = stage_ap.rearrange("(n p) o -> p n o", p=128)
            out_r = out_ap.rearrange("(n p) o -> p n o", p=128)
            for i in range(n_chunks):
                q = q_pool.tile([128, CHUNK, out_per_core], F32, tag="q")
                nc.sync.dma_start(q, stage_r[:, i * CHUNK:(i + 1) * CHUNK, :])
                nc.vector.tensor_scalar(
                    q, q, sq_b[:, 0:1], RND_C,
                    mybir.AluOpType.mult, mybir.AluOpType.add,
                )
                nc.vector.tensor_scalar(
                    q, q, RND_C, sq_b[:, 1:2],
                    mybir.AluOpType.subtract, mybir.AluOpType.mult,
                )
                nc.sync.dma_start(out_r[:, i * CHUNK:(i + 1) * CHUNK, :], q)

    nc.compile()
    return nc


_NC_CACHE: dict = {}


def _get_full_nc():
    key = "full"
    if key not in _NC_CACHE:
        _NC_CACHE[key] = build_kernel(
            tok_per_core=(B * S) // GRID_R,
            k_dim=D_IN,
            out_per_core=D_OUT // GRID_C,
            tok_block=512,
            n_weight_copies=GRID_R,
            debug=False,
        )
    return _NC_CACHE[key]


def make_in_maps(x, weight, bias, scale, grid_r=GRID_R, grid_c=GRID_C, tok_block=512):
    """Host-side layout prep: transpose/cast/shard. No arithmetic on values."""
    x = np.asarray(x, dtype=np.float32)
    weight = np.asarray(weight, dtype=np.float32)
    bias = np.asarray(bias, dtype=np.float32)
    scale = np.asarray(scale, dtype=np.float32)

    n_tok = x.size // x.shape[-1]
    k_dim = x.shape[-1]
    d_out = weight.shape[0]
    tok_pc = n_tok // grid_r
    out_pc = d_out // grid_c
    n_blocks = tok_pc // tok_block

    xf = x.reshape(n_tok, k_dim)
    # [k, n_tok] bf16 (single transpose+cast pass)
    xtb = xf.T.astype(ml_dtypes.bfloat16)
    wt_full = np.ascontiguousarray(weight.T)  # [k, d_out]

    in_maps = []
    for cid in range(grid_r * grid_c):
        r, c = divmod(cid, grid_c)
        xs = xtb[:, r * tok_pc:(r + 1) * tok_pc]  # [k, tok_pc]
        # -> [n_blocks, k, tok_block]
        xs = np.ascontiguousarray(
            xs.reshape(k_dim, n_blocks, tok_block).transpose(1, 0, 2)
        )
        in_maps.append(
            {
                "xt": xs,
                "wt": np.ascontiguousarray(wt_full[:, c * out_pc:(c + 1) * out_pc]),
                "biasv": np.ascontiguousarray(bias[c * out_pc:(c + 1) * out_pc]),
                "scalev": scale.reshape(1),
            }
        )
    return in_maps


def assemble_out(results, out_shape, grid_r=GRID_R, grid_c=GRID_C):
    n_tok = int(np.prod(out_shape[:-1]))
    d_out = out_shape[-1]
    tok_pc = n_tok // grid_r
    out_pc = d_out // grid_c
    full = np.empty((n_tok, d_out), dtype=np.float32)
    for cid in range(grid_r * grid_c):
        r, c = divmod(cid, grid_c)
        full[r * tok_pc:(r + 1) * tok_pc, c * out_pc:(c + 1) * out_pc] = results[cid][
            "outv"
        ]
    return full.reshape(out_shape)


def kernel(x, weight, bias, scale):
    nc = _get_full_nc()
    in_maps = make_in_maps(x, weight, bias, scale)
    res = bass_utils.run_bass_kernel_spmd(nc, in_maps, core_ids=list(range(N_CORES)))
    return assemble_out(res.results, (B, S, D_OUT))


if __name__ == "__main__":
    import reference

    inputs = reference.setup_inputs()
    out = kernel(**{k: np.asarray(v) for k, v in inputs.items()})
    print("kernel out", out.shape, out.dtype)


# revision 6
# speedup vs baseline: 1.0367x; 1.0367x over previous
"""BitLinear (ternary-weight linear + global activation requant) on 8 TRN2 cores.

Computation (see reference):
    wq  = ternarize(weight * scale, thr = 0.7*mean|weight*scale|)   # {-1,0,+1}
    out = x @ wq.T + bias
    s   = 255 / (max(out) - min(out));  out = round(out*s)/s

Sharding: 2x4 grid over (tokens, out_features).  Each core computes a
[4096 tok, 1024 out] shard contracting over the full K=4096.
x is pre-transposed/cast to bf16 on the host (layout work only); the ternary
threshold and the activation max/min are reduced across cores on-device with
two tiny AllReduces, exactly as the reference math requires.
"""

import numpy as np
import ml_dtypes

import concourse.bass as bass
import concourse.mybir as mybir
import concourse.tile as tile
from concourse import bacc
from concourse import bass_utils

F32 = mybir.dt.float32
BF16 = mybir.dt.bfloat16

# Full problem shape
B, S, D_IN, D_OUT = 4, 2048, 4096, 4096
N_CORES = 8
GRID_R, GRID_C = 2, 4  # token shards x out-feature shards

# Round-to-nearest-even magic constant (valid for |y| < 2^22)
RND_C = float(np.float32(12582912.0))  # 1.5 * 2^23


def build_kernel(
    tok_per_core: int,
    k_dim: int,
    out_per_core: int,
    tok_block: int,
    n_weight_copies: int,
    debug: bool = False,
    repeat: int = 1,
):
    """Build + compile the per-core SPMD Bass program.

    n_weight_copies: how many cores hold the same W shard (= GRID_R); the
    threshold AllReduce over all cores over-counts by this factor.
    """
    KO = k_dim // 128
    SUBS = tok_block // 128
    OGS = max(1, out_per_core // 512)
    OGW = min(512, out_per_core)  # o-group width
    N_BLOCKS = tok_per_core // tok_block
    assert KO * 128 == k_dim and SUBS * 128 == tok_block
    assert OGS * OGW == out_per_core and N_BLOCKS * tok_block == tok_per_core

    nc = bacc.Bacc(
        "TRN2",
        target_bir_lowering=False,
        debug=debug,
        enable_asserts=False,
        num_devices=N_CORES,
    )

    xt = nc.declare_dram_parameter("xt", [N_BLOCKS, k_dim, tok_block], BF16, isOutput=False)
    wt = nc.declare_dram_parameter("wt", [k_dim, out_per_core], F32, isOutput=False)
    biasv = nc.declare_dram_parameter("biasv", [out_per_core], F32, isOutput=False)
    scalev = nc.declare_dram_parameter("scalev", [1], F32, isOutput=False)
    out = nc.declare_dram_parameter("outv", [tok_per_core, out_per_core], F32, isOutput=True)

    # raw (pre-quant) output staging in DRAM
    stage = nc.dram_tensor("stage", [tok_per_core, out_per_core], F32)

    xt_ap = xt.ap()
    wt_ap = wt.ap()
    stage_ap = stage.ap()
    out_ap = out.ap()

    n_drains = N_BLOCKS * SUBS * OGS

    with tile.TileContext(nc) as tc:
        with (
            tc.tile_pool(name="const", bufs=1) as const_pool,
            tc.tile_pool(name="wslab", bufs=5) as wslab_pool,
            tc.tile_pool(name="wq", bufs=1) as wq_pool,
            tc.tile_pool(name="xbuf", bufs=2) as x_pool,
            tc.tile_pool(name="drain", bufs=3) as drain_pool,
            tc.tile_pool(name="qt", bufs=2) as q_pool,
            tc.tile_pool(name="psum", bufs=1, space="PSUM") as psum_pool,
            tc.tile_pool(name="dram", bufs=1, space="DRAM") as dram_pool,
        ):
          for _rep in range(repeat):
            # ---- tiny constants --------------------------------------------
            scale_sb = const_pool.tile([1, 1], F32, tag="scale_sb")
            nc.sync.dma_start(scale_sb, scalev.ap()[None, :])
            scale_b = const_pool.tile([128, 1], F32, tag="scale_b")
            nc.gpsimd.partition_broadcast(scale_b, scale_sb)

            bias_sb = const_pool.tile([1, out_per_core], F32, tag="bias_sb")
            nc.sync.dma_start(bias_sb, biasv.ap()[None, :])
            bias_b = const_pool.tile([128, out_per_core], F32, tag="bias_b")
            nc.gpsimd.partition_broadcast(bias_b, bias_sb)

            # ---- phase 1: |W*scale| sum + AllReduce -> ternary threshold ---
            wsum = const_pool.tile([128, KO], F32, tag="wsum")
            for ko in range(KO):
                wslab = wslab_pool.tile([128, out_per_core], F32, tag="wslab")
                nc.sync.dma_start(wslab, wt_ap[ko * 128:(ko + 1) * 128, :])
                ws = wslab_pool.tile([128, out_per_core], F32, tag="wslab")
                nc.vector.tensor_scalar_mul(ws, wslab, scale_b)
                nc.vector.tensor_reduce(
                    wsum[:, ko:ko + 1], ws, axis=mybir.AxisListType.X,
                    op=mybir.AluOpType.add, apply_absolute_value=True,
                )

            wsum1 = const_pool.tile([128, 1], F32, tag="wsum1")
            nc.vector.tensor_reduce(
                wsum1, wsum, axis=mybir.AxisListType.X, op=mybir.AluOpType.add
            )
            wsum_all = const_pool.tile([128, 1], F32, tag="wsum_all")
            nc.gpsimd.partition_all_reduce(
                wsum_all, wsum1, 128, bass.bass_isa.ReduceOp.add
            )

            cc1_in = dram_pool.tile([1, 1], F32)
            cc1_out = dram_pool.tile([1, 1], F32)
            nc.sync.dma_start(cc1_in, wsum_all[0:1, :])
            nc.gpsimd.collective_compute(
                "AllReduce",
                mybir.AluOpType.add,
                replica_groups=[list(range(N_CORES))],
                ins=[cc1_in.opt()],
                outs=[cc1_out.opt()],
            )
            s_glob = const_pool.tile([1, 1], F32, tag="s_glob")
            nc.sync.dma_start(s_glob, cc1_out)

            # thr2 = [t, -t];  t = 0.7 * (S_global / n_copies) / (k_dim*D_OUT)
            n_w_elems = float(k_dim * GRID_C * out_per_core)
            tcoef = float(np.float32(0.7) / np.float64(n_weight_copies * n_w_elems))
            thr_c = const_pool.tile([1, 2], F32, tag="thr_c")
            nc.vector.memset(thr_c[:, 0:1], tcoef)
            nc.vector.memset(thr_c[:, 1:2], -tcoef)
            thr2 = const_pool.tile([1, 2], F32, tag="thr2")
            nc.vector.tensor_scalar_mul(thr2, thr_c, s_glob)
            thr_b = const_pool.tile([128, 2], F32, tag="thr_b")
            nc.gpsimd.partition_broadcast(thr_b, thr2)

            # ---- phase 1b: ternarize W -> resident bf16 wq [128, KO, out] --
            wq = wq_pool.tile([128, KO, out_per_core], BF16, tag="wq")
            for ko in range(KO):
                wslab = wslab_pool.tile([128, out_per_core], F32, tag="wslab")
                nc.sync.dma_start(wslab, wt_ap[ko * 128:(ko + 1) * 128, :])
                ws = wslab_pool.tile([128, out_per_core], F32, tag="wslab")
                nc.vector.tensor_scalar_mul(ws, wslab, scale_b)
                g = wslab_pool.tile([128, out_per_core], BF16, tag="tern_g")
                l = wslab_pool.tile([128, out_per_core], BF16, tag="tern_l")
                nc.vector.tensor_scalar(
                    g, ws, thr_b[:, 0:1], None, mybir.AluOpType.is_gt
                )
                nc.vector.tensor_scalar(
                    l, ws, thr_b[:, 1:2], None, mybir.AluOpType.is_lt
                )
                nc.vector.tensor_sub(wq[:, ko, :], g, l)

            # ---- phase 2: matmul blocks ------------------------------------
            maxst = const_pool.tile([128, n_drains], F32, tag="maxst")
            minst = const_pool.tile([128, n_drains], F32, tag="minst")

            for blk in range(N_BLOCKS):
                x_tile = x_pool.tile([128, KO, tok_block], BF16, tag="x_tile")
                nc.sync.dma_start(
                    x_tile, xt_ap[blk].rearrange("(ko p) t -> p ko t", p=128)
                )
                psums = [
                    [
                        psum_pool.tile([128, OGW], F32, name=f"ps_{sub}_{og}")
                        for og in range(OGS)
                    ]
                    for sub in range(SUBS)
                ]
                for ko in range(KO):
                    for sub in range(SUBS):
                        lhsT = x_tile[:, ko, sub * 128:(sub + 1) * 128]
                        for og in range(OGS):
                            nc.tensor.matmul(
                                psums[sub][og],
                                lhsT,
                                wq[:, ko, og * OGW:(og + 1) * OGW],
                                start=(ko == 0),
                                stop=(ko == KO - 1),
                            )
                # drain: +bias, track max/min, stage raw fp32 to DRAM
                for sub in range(SUBS):
                    tok0 = blk * tok_block + sub * 128
                    for og in range(OGS):
                        d = drain_pool.tile([128, OGW], F32, tag="drain")
                        nc.vector.tensor_add(
                            d, psums[sub][og], bias_b[:, og * OGW:(og + 1) * OGW]
                        )
                        idx = (blk * SUBS + sub) * OGS + og
                        nc.vector.tensor_reduce(
                            maxst[:, idx:idx + 1], d, axis=mybir.AxisListType.X,
                            op=mybir.AluOpType.max,
                        )
                        nc.vector.tensor_reduce(
                            minst[:, idx:idx + 1], d, axis=mybir.AxisListType.X,
                            op=mybir.AluOpType.min,
                        )
                        nc.sync.dma_start(
                            stage_ap[tok0:tok0 + 128, og * OGW:(og + 1) * OGW], d
                        )

            # ---- phase 3: global max/min -> s -> requantize ----------------
            lmax = const_pool.tile([128, 1], F32, tag="lmax")
            lmin = const_pool.tile([128, 1], F32, tag="lmin")
            nc.vector.tensor_reduce(
                lmax, maxst, axis=mybir.AxisListType.X, op=mybir.AluOpType.max
            )
            nc.vector.tensor_reduce(
                lmin, minst, axis=mybir.AxisListType.X, op=mybir.AluOpType.min
            )
            st2 = const_pool.tile([128, 2], F32, tag="st2")
            nc.vector.tensor_copy(out=st2[:, 0:1], in_=lmax)
            nc.vector.tensor_scalar_mul(st2[:, 1:2], lmin, -1.0)
            st2r = const_pool.tile([128, 2], F32, tag="st2r")
            nc.gpsimd.partition_all_reduce(
                st2r, st2, 128, bass.bass_isa.ReduceOp.max
            )

            cc2_in = dram_pool.tile([1, 2], F32)
            cc2_out = dram_pool.tile([1, 2], F32)
            nc.sync.dma_start(cc2_in, st2r[0:1, :])
            nc.gpsimd.collective_compute(
                "AllReduce",
                mybir.AluOpType.max,
                replica_groups=[list(range(N_CORES))],
                ins=[cc2_in.opt()],
                outs=[cc2_out.opt()],
            )
            gst = const_pool.tile([1, 2], F32, tag="gst")
            nc.sync.dma_start(gst, cc2_out)

            rng = const_pool.tile([1, 1], F32, tag="rng")  # max - min
            nc.vector.tensor_reduce(
                rng, gst, axis=mybir.AxisListType.X, op=mybir.AluOpType.add
            )

            def accurate_recip(out_ap, in_ap, tag):
                # r1 = r0*(2 - x*r0), one Newton step on InstReciprocal
                r0 = const_pool.tile([1, 1], F32, tag=f"{tag}_r0")
                nc.vector.reciprocal(r0, in_ap)
                e = const_pool.tile([1, 1], F32, tag=f"{tag}_e")
                nc.vector.tensor_scalar(
                    e, in_ap, r0, None, mybir.AluOpType.mult
                )
                nc.vector.tensor_scalar(
                    e, e, -1.0, 2.0, mybir.AluOpType.mult, mybir.AluOpType.add
                )
                nc.vector.tensor_mul(out_ap, r0, e)

            sq = const_pool.tile([1, 2], F32, tag="sq")  # [s, 1/s]
            rinv = const_pool.tile([1, 1], F32, tag="rinv")
            accurate_recip(rinv, rng, "rr")
            nc.vector.tensor_scalar_mul(sq[:, 0:1], rinv, 255.0)
            accurate_recip(sq[:, 1:2], sq[:, 0:1], "si")
            sq_b = const_pool.tile([128, 2], F32, tag="sq_b")
            nc.gpsimd.partition_broadcast(sq_b, sq)

            # requantize: q = (round(y*s))/s with RNE via +/- 1.5*2^23
            CHUNK = 2  # 128-row groups per quantize tile
            n_chunks = (tok_per_core // 128) // CHUNK
            stage_r = stage_ap.rearrange("(n p) o -> p n o", p=128)
            out_r = out_ap.rearrange("(n p) o -> p n o", p=128)
            for i in range(n_chunks):
                q = q_pool.tile([128, CHUNK, out_per_core], F32, tag="q")
                nc.sync.dma_start(q, stage_r[:, i * CHUNK:(i + 1) * CHUNK, :])
                nc.vector.tensor_scalar(
                    q, q, sq_b[:, 0:1], RND_C,
                    mybir.AluOpType.mult, mybir.AluOpType.add,
                )
                nc.vector.tensor_scalar(
                    q, q, RND_C, sq_b[:, 1:2],
                    mybir.AluOpType.subtract, mybir.AluOpType.mult,
                )
                nc.sync.dma_start(out_r[:, i * CHUNK:(i + 1) * CHUNK, :], q)

    nc.compile()
    return nc


_NC_CACHE: dict = {}


def _get_full_nc():
    key = "full"
    if key not in _NC_CACHE:
        _NC_CACHE[key] = build_kernel(
            tok_per_core=(B * S) // GRID_R,
            k_dim=D_IN,
            out_per_core=D_OUT // GRID_C,
            tok_block=512,
            n_weight_copies=GRID_R,
            debug=False,
        )
    return _NC_CACHE[key]


def make_in_maps(x, weight, bias, scale, grid_r=GRID_R, grid_c=GRID_C, tok_block=512):
    """Host-side layout prep: transpose/cast/shard. No arithmetic on values."""
    x = np.asarray(x, dtype=np.float32)
    weight = np.asarray(weight, dtype=np.float32)
    bias = np.asarray(bias, dtype=np.float32)
    scale = np.asarray(scale, dtype=np.float32)

    n_tok = x.size // x.shape[-1]
    k_dim = x.shape[-1]
    d_out = weight.shape[0]
    tok_pc = n_tok // grid_r
    out_pc = d_out // grid_c
    n_blocks = tok_pc // tok_block

    xf = x.reshape(n_tok, k_dim)
    # [k, n_tok] bf16 (single transpose+cast pass)
    xtb = xf.T.astype(ml_dtypes.bfloat16)
    wt_full = np.ascontiguousarray(weight.T)  # [k, d_out]

    in_maps = []
    for cid in range(grid_r * grid_c):
        r, c = divmod(cid, grid_c)
        xs = xtb[:, r * tok_pc:(r + 1) * tok_pc]  # [k, tok_pc]
        # -> [n_blocks, k, tok_block]
        xs = np.ascontiguousarray(
            xs.reshape(k_dim, n_blocks, tok_block).transpose(1, 0, 2)
        )
        in_maps.append(
            {
                "xt": xs,
                "wt": np.ascontiguousarray(wt_full[:, c * out_pc:(c + 1) * out_pc]),
                "biasv": np.ascontiguousarray(bias[c * out_pc:(c + 1) * out_pc]),
                "scalev": scale.reshape(1),
            }
        )
    return in_maps


def assemble_out(results, out_shape, grid_r=GRID_R, grid_c=GRID_C):
    n_tok = int(np.prod(out_shape[:-1]))
    d_out = out_shape[-1]
    tok_pc = n_tok // grid_r
    out_pc = d_out // grid_c
    full = np.empty((n_tok, d_out), dtype=np.float32)
    for cid in range(grid_r * grid_c):
        r, c = divmod(cid, grid_c)
        full[r * tok_pc:(r + 1) * tok_pc, c * out_pc:(c + 1) * out_pc] = results[cid][
            "outv"
        ]
    return full.reshape(out_shape)


def kernel(x, weight, bias, scale):
    nc = _get_full_nc()
    in_maps = make_in_maps(x, weight, bias, scale)
    res = bass_utils.run_bass_kernel_spmd(nc, in_maps, core_ids=list(range(N_CORES)))
    return assemble_out(res.results, (B, S, D_OUT))


if __name__ == "__main__":
    import reference

    inputs = reference.setup_inputs()
    out = kernel(**{k: np.asarray(v) for k, v in inputs.items()})
    print("kernel out", out.shape, out.dtype)


# revision 18
# speedup vs baseline: 1.2039x; 1.1614x over previous
"""BitLinear (ternary-weight linear + global activation requant) on 8 TRN2 cores.

Computation (see reference):
    wq  = ternarize(weight * scale, thr = 0.7*mean|weight*scale|)   # {-1,0,+1}
    out = x @ wq.T + bias
    s   = 255 / (max(out) - min(out));  out = round(out*s)/s

Sharding: 2x4 grid over (tokens, out_features).  Each core computes a
[4096 tok, 1024 out] shard contracting over the full K=4096.
x is pre-transposed/cast to bf16 on the host (layout work only); the ternary
threshold and the activation max/min are reduced across cores on-device with
two tiny AllReduces, exactly as the reference math requires.
"""

import numpy as np
import ml_dtypes

import concourse.bass as bass
import concourse.mybir as mybir
import concourse.tile as tile
from concourse import bacc
from concourse import bass_utils

F32 = mybir.dt.float32
BF16 = mybir.dt.bfloat16
F16 = mybir.dt.float16

# Full problem shape
B, S, D_IN, D_OUT = 4, 2048, 4096, 4096
N_CORES = 8
GRID_R, GRID_C = 2, 4  # token shards x out-feature shards

# Round-to-nearest-even magic constant (valid for |y| < 2^22)
RND_C = float(np.float32(12582912.0))  # 1.5 * 2^23


def build_kernel(
    tok_per_core: int,
    k_dim: int,
    out_per_core: int,
    tok_block: int,
    n_weight_copies: int,
    debug: bool = False,
    repeat: int = 1,
    rep_w: int = 1,
    rep_mm: int = 1,
    rep_tail: int = 1,
    use_collectives: bool = True,
    thr_collective: bool = True,
    stage_f16: bool = False,
    mm_no_drain: bool = False,
    mm_share_x: bool = False,
):
    """Build + compile the per-core SPMD Bass program.

    rep_* repeat individual phases in-NEFF (timing instrumentation only;
    results are unchanged since repeated phases recompute identical data).
    """
    KO = k_dim // 128
    SUBS = tok_block // 128
    OGS = max(1, out_per_core // 512)
    OGW = min(512, out_per_core)  # o-group width
    N_BLOCKS = tok_per_core // tok_block
    assert KO * 128 == k_dim and SUBS * 128 == tok_block
    assert OGS * OGW == out_per_core and N_BLOCKS * tok_block == tok_per_core

    nc = bacc.Bacc(
        "TRN2",
        target_bir_lowering=False,
        debug=debug,
        enable_asserts=False,
        num_devices=N_CORES,
    )

    xt = nc.declare_dram_parameter("xt", [N_BLOCKS, k_dim, tok_block], BF16, isOutput=False)
    wt = nc.declare_dram_parameter("wt", [k_dim, out_per_core], F32, isOutput=False)
    d_out_total = out_per_core * GRID_C
    wtb_cols = out_per_core if thr_collective else d_out_total
    wtb = nc.declare_dram_parameter("wtb", [k_dim, wtb_cols], BF16, isOutput=False)
    biasv = nc.declare_dram_parameter("biasv", [out_per_core], F32, isOutput=False)
    scalev = nc.declare_dram_parameter("scalev", [1], F32, isOutput=False)
    out = nc.declare_dram_parameter("outv", [tok_per_core, out_per_core], F32, isOutput=True)

    # raw (pre-quant) output staging in DRAM
    SDT = F16 if stage_f16 else F32
    stage = nc.dram_tensor("stage", [tok_per_core, out_per_core], SDT)

    xt_ap = xt.ap()
    wt_ap = wt.ap()
    wtb_ap = wtb.ap()
    stage_ap = stage.ap()
    out_ap = out.ap()

    n_drains = N_BLOCKS * SUBS * OGS

    with tile.TileContext(nc) as tc:
        with (
            tc.tile_pool(name="const", bufs=1) as const_pool,
            tc.tile_pool(name="wslab", bufs=4) as wslab_pool,
            tc.tile_pool(name="wq", bufs=1) as wq_pool,
            tc.tile_pool(name="xbuf", bufs=2) as x_pool,
            tc.tile_pool(name="drain", bufs=3) as drain_pool,
            tc.tile_pool(name="qt", bufs=2) as q_pool,
            tc.tile_pool(name="psum", bufs=1, space="PSUM") as psum_pool,
            tc.tile_pool(name="dram", bufs=1, space="DRAM") as dram_pool,
        ):

            def phase_consts():
                scale_sb = const_pool.tile([1, 1], F32, tag="scale_sb")
                nc.sync.dma_start(scale_sb, scalev.ap()[None, :])
                scale_b = const_pool.tile([128, 1], F32, tag="scale_b")
                nc.gpsimd.partition_broadcast(scale_b, scale_sb)

                bias_sb = const_pool.tile([1, out_per_core], F32, tag="bias_sb")
                nc.sync.dma_start(bias_sb, biasv.ap()[None, :])
                bias_b = const_pool.tile([128, out_per_core], F32, tag="bias_b")
                nc.gpsimd.partition_broadcast(bias_b, bias_sb)
                return scale_sb, scale_b, bias_b

            def phase_w(scale_sb, scale_b):
                """|W|*|scale| global mean -> threshold -> ternarize to bf16.

                The abs-sum runs on a bf16 copy of W (half the DMA, RNE
                rounding is mean-neutral); ternarize compares use fp32 W.
                """
                n_csh = 1 if thr_collective else GRID_C
                wsum = const_pool.tile([128, KO * n_csh], F32, tag="wsum")
                for ko in range(KO):
                    for csh in range(n_csh):
                        c0 = (0 if thr_collective else csh) * out_per_core
                        wb = wslab_pool.tile(
                            [128, out_per_core], BF16, tag="wbslab", bufs=3
                        )
                        nc.sync.dma_start(
                            wb,
                            wtb_ap[ko * 128:(ko + 1) * 128, c0:c0 + out_per_core],
                        )
                        nc.vector.tensor_reduce(
                            wsum[:, ko * n_csh + csh:ko * n_csh + csh + 1], wb,
                            axis=mybir.AxisListType.X,
                            op=mybir.AluOpType.add, apply_absolute_value=True,
                        )

                wsum1 = const_pool.tile([128, 1], F32, tag="wsum1")
                nc.vector.tensor_reduce(
                    wsum1, wsum, axis=mybir.AxisListType.X, op=mybir.AluOpType.add
                )
                wsum_all = const_pool.tile([128, 1], F32, tag="wsum_all")
                nc.gpsimd.partition_all_reduce(
                    wsum_all, wsum1, 128, bass.bass_isa.ReduceOp.add
                )

                if thr_collective:
                    cc1_in = dram_pool.tile([1, 1], F32, tag="cc1_in")
                    cc1_out = dram_pool.tile([1, 1], F32, tag="cc1_out")
                    nc.sync.dma_start(cc1_in, wsum_all[0:1, :])
                    if use_collectives:
                        nc.gpsimd.collective_compute(
                            "AllReduce",
                            mybir.AluOpType.add,
                            replica_groups=[list(range(N_CORES))],
                            ins=[cc1_in.opt()],
                            outs=[cc1_out.opt()],
                        )
                    else:
                        nc.sync.dma_start(cc1_out, cc1_in)
                    s_glob = const_pool.tile([1, 1], F32, tag="s_glob")
                    nc.sync.dma_start(s_glob, cc1_out)
                else:
                    s_glob = wsum_all[0:1, :]

                # thr2 = [t, -t];  t = 0.7 * (S_global/n_copies) / n_elems(W)
                n_w_elems = float(k_dim * GRID_C * out_per_core)
                n_cp = n_weight_copies * N_CORES // (GRID_R * GRID_C) if thr_collective else 1
                tcoef = float(np.float32(0.7) / np.float64(n_cp * n_w_elems))
                thr_c = const_pool.tile([1, 2], F32, tag="thr_c")
                nc.vector.memset(thr_c[:, 0:1], tcoef)
                nc.vector.memset(thr_c[:, 1:2], -tcoef)
                absscale = const_pool.tile([1, 1], F32, tag="absscale")
                nc.vector.tensor_reduce(
                    absscale, scale_sb, axis=mybir.AxisListType.X,
                    op=mybir.AluOpType.max, apply_absolute_value=True,
                )
                thr2 = const_pool.tile([1, 2], F32, tag="thr2")
                nc.vector.tensor_scalar_mul(thr2, thr_c, s_glob)
                nc.vector.tensor_scalar_mul(thr2, thr2, absscale)
                thr_b = const_pool.tile([128, 2], F32, tag="thr_b")
                nc.gpsimd.partition_broadcast(thr_b, thr2)

                wq = wq_pool.tile([128, KO, out_per_core], BF16, tag="wq")
                for ko in range(KO):
                    wslab = wslab_pool.tile([128, out_per_core], F32, tag="wslab")
                    nc.sync.dma_start(wslab, wt_ap[ko * 128:(ko + 1) * 128, :])
                    ws = wslab_pool.tile([128, out_per_core], F32, tag="wslab")
                    nc.vector.tensor_scalar_mul(ws, wslab, scale_b)
                    g = wslab_pool.tile([128, out_per_core], BF16, tag="tern_g", bufs=3)
                    l = wslab_pool.tile([128, out_per_core], BF16, tag="tern_l", bufs=3)
                    nc.vector.tensor_scalar(
                        g, ws, thr_b[:, 0:1], None, mybir.AluOpType.is_gt
                    )
                    nc.vector.tensor_scalar(
                        l, ws, thr_b[:, 1:2], None, mybir.AluOpType.is_lt
                    )
                    nc.vector.tensor_sub(wq[:, ko, :], g, l)
                return wq

            def phase_mm(wq, bias_b):
                """Matmul blocks: accumulate K in PSUM, +bias, max/min, stage."""
                maxst = const_pool.tile([128, n_drains], F32, tag="maxst")
                minst = const_pool.tile([128, n_drains], F32, tag="minst")
                if mm_no_drain:  # timing-only variant: stats never written
                    nc.vector.memset(maxst, 1.0)
                    nc.vector.memset(minst, -1.0)

                for blk in range(N_BLOCKS):
                    if mm_share_x and blk > 0:
                        pass  # timing-only: reuse previous x_tile
                    else:
                        x_tile = x_pool.tile([128, KO, tok_block], BF16, tag="x_tile")
                        nc.sync.dma_start(
                            x_tile, xt_ap[blk].rearrange("(ko p) t -> p ko t", p=128)
                        )
                    psums = [
                        [
                            psum_pool.tile([128, OGW], F32, name=f"ps_{sub}_{og}")
                            for og in range(OGS)
                        ]
                        for sub in range(SUBS)
                    ]
                    for ko in range(KO):
                        for sub in range(SUBS):
                            lhsT = x_tile[:, ko, sub * 128:(sub + 1) * 128]
                            for og in range(OGS):
                                nc.tensor.matmul(
                                    psums[sub][og],
                                    lhsT,
                                    wq[:, ko, og * OGW:(og + 1) * OGW],
                                    start=(ko == 0),
                                    stop=(ko == KO - 1),
                                )
                    ds = []
                    for sub in range(SUBS):
                        for og in range(OGS):
                            d = drain_pool.tile([128, OGW], SDT, tag="drain", bufs=10)
                            nc.vector.tensor_add(
                                d, psums[sub][og], bias_b[:, og * OGW:(og + 1) * OGW]
                            )
                            ds.append((sub, og, d))
                    for sub, og, d in ds:
                        tok0 = blk * tok_block + sub * 128
                        if not mm_no_drain:
                            idx = (blk * SUBS + sub) * OGS + og
                            nc.vector.tensor_reduce(
                                maxst[:, idx:idx + 1], d, axis=mybir.AxisListType.X,
                                op=mybir.AluOpType.max,
                            )
                            nc.vector.tensor_reduce(
                                minst[:, idx:idx + 1], d, axis=mybir.AxisListType.X,
                                op=mybir.AluOpType.min,
                            )
                        nc.sync.dma_start(
                            stage_ap[tok0:tok0 + 128, og * OGW:(og + 1) * OGW], d
                        )
                return maxst, minst

            def phase_tail(maxst, minst):
                """Global max/min -> s -> requantize staged output."""
                lmax = const_pool.tile([128, 1], F32, tag="lmax")
                lmin = const_pool.tile([128, 1], F32, tag="lmin")
                nc.vector.tensor_reduce(
                    lmax, maxst, axis=mybir.AxisListType.X, op=mybir.AluOpType.max
                )
                nc.vector.tensor_reduce(
                    lmin, minst, axis=mybir.AxisListType.X, op=mybir.AluOpType.min
                )
                st2 = const_pool.tile([128, 2], F32, tag="st2")
                nc.vector.tensor_copy(out=st2[:, 0:1], in_=lmax)
                nc.vector.tensor_scalar_mul(st2[:, 1:2], lmin, -1.0)
                st2r = const_pool.tile([128, 2], F32, tag="st2r")
                nc.gpsimd.partition_all_reduce(
                    st2r, st2, 128, bass.bass_isa.ReduceOp.max
                )

                cc2_in = dram_pool.tile([1, 2], F32, tag="cc2_in")
                cc2_out = dram_pool.tile([1, 2], F32, tag="cc2_out")
                nc.sync.dma_start(cc2_in, st2r[0:1, :])
                if use_collectives:
                    nc.gpsimd.collective_compute(
                        "AllReduce",
                        mybir.AluOpType.max,
                        replica_groups=[list(range(N_CORES))],
                        ins=[cc2_in.opt()],
                        outs=[cc2_out.opt()],
                    )
                else:
                    nc.sync.dma_start(cc2_out, cc2_in)
                gst = const_pool.tile([1, 2], F32, tag="gst")
                nc.sync.dma_start(gst, cc2_out)

                rng = const_pool.tile([1, 1], F32, tag="rng")  # max - min
                nc.vector.tensor_reduce(
                    rng, gst, axis=mybir.AxisListType.X, op=mybir.AluOpType.add
                )

                def accurate_recip(out_ap2, in_ap, tag):
                    # r1 = r0*(2 - x*r0), one Newton step on InstReciprocal
                    r0 = const_pool.tile([1, 1], F32, tag=f"{tag}_r0")
                    nc.vector.reciprocal(r0, in_ap)
                    e = const_pool.tile([1, 1], F32, tag=f"{tag}_e")
                    nc.vector.tensor_scalar(
                        e, in_ap, r0, None, mybir.AluOpType.mult
                    )
                    nc.vector.tensor_scalar(
                        e, e, -1.0, 2.0, mybir.AluOpType.mult, mybir.AluOpType.add
                    )
                    nc.vector.tensor_mul(out_ap2, r0, e)

                sq = const_pool.tile([1, 2], F32, tag="sq")  # [s, 1/s]
                rinv = const_pool.tile([1, 1], F32, tag="rinv")
                accurate_recip(rinv, rng, "rr")
                nc.vector.tensor_scalar_mul(sq[:, 0:1], rinv, 255.0)
                accurate_recip(sq[:, 1:2], sq[:, 0:1], "si")
                sq_b = const_pool.tile([128, 2], F32, tag="sq_b")
                nc.gpsimd.partition_broadcast(sq_b, sq)

                # q = round(y*s)/s with RNE via +/- 1.5*2^23
                CHUNK = 1  # 128-row groups per quantize tile
                n_chunks = (tok_per_core // 128) // CHUNK
                stage_r = stage_ap.rearrange("(n p) o -> p n o", p=128)
                out_r = out_ap.rearrange("(n p) o -> p n o", p=128)
                for i in range(n_chunks):
                    q = q_pool.tile([128, CHUNK, out_per_core], F32, tag="q", bufs=3)
                    if stage_f16:
                        qh = q_pool.tile(
                            [128, CHUNK, out_per_core], SDT, tag="qh", bufs=3
                        )
                        nc.sync.dma_start(qh, stage_r[:, i * CHUNK:(i + 1) * CHUNK, :])
                    else:
                        qh = q
                        nc.sync.dma_start(q, stage_r[:, i * CHUNK:(i + 1) * CHUNK, :])
                    nc.vector.tensor_scalar(
                        q, qh, sq_b[:, 0:1], RND_C,
                        mybir.AluOpType.mult, mybir.AluOpType.add,
                    )
                    nc.vector.tensor_scalar(
                        q, q, RND_C, sq_b[:, 1:2],
                        mybir.AluOpType.subtract, mybir.AluOpType.mult,
                    )
                    nc.sync.dma_start(out_r[:, i * CHUNK:(i + 1) * CHUNK, :], q)

            for _ in range(repeat):
                scale_sb, scale_b, bias_b = phase_consts()
                for _ in range(rep_w):
                    wq = phase_w(scale_sb, scale_b)
                for _ in range(rep_mm):
                    maxst, minst = phase_mm(wq, bias_b)
                for _ in range(rep_tail):
                    phase_tail(maxst, minst)

    nc.compile()
    return nc


_NC_CACHE: dict = {}


def _get_full_nc():
    key = "full"
    if key not in _NC_CACHE:
        _NC_CACHE[key] = build_kernel(
            tok_per_core=(B * S) // GRID_R,
            k_dim=D_IN,
            out_per_core=D_OUT // GRID_C,
            tok_block=512,
            n_weight_copies=GRID_R,
            debug=False,
        )
    return _NC_CACHE[key]


def make_in_maps(x, weight, bias, scale, grid_r=GRID_R, grid_c=GRID_C,
                 tok_block=512, thr_collective=True):
    """Host-side layout prep: transpose/cast/shard. No arithmetic on values."""
    x = np.asarray(x, dtype=np.float32)
    weight = np.asarray(weight, dtype=np.float32)
    bias = np.asarray(bias, dtype=np.float32)
    scale = np.asarray(scale, dtype=np.float32)

    n_tok = x.size // x.shape[-1]
    k_dim = x.shape[-1]
    d_out = weight.shape[0]
    tok_pc = n_tok // grid_r
    out_pc = d_out // grid_c
    n_blocks = tok_pc // tok_block

    xf = x.reshape(n_tok, k_dim)
    # [k, n_tok] bf16 (single transpose+cast pass)
    xtb = xf.T.astype(ml_dtypes.bfloat16)
    wt_full = np.ascontiguousarray(weight.T)  # [k, d_out]
    wtb_full = wt_full.astype(ml_dtypes.bfloat16)

    in_maps = []
    for cid in range(grid_r * grid_c):
        r, c = divmod(cid, grid_c)
        xs = xtb[:, r * tok_pc:(r + 1) * tok_pc]  # [k, tok_pc]
        # -> [n_blocks, k, tok_block]
        xs = np.ascontiguousarray(
            xs.reshape(k_dim, n_blocks, tok_block).transpose(1, 0, 2)
        )
        in_maps.append(
            {
                "xt": xs,
                "wt": np.ascontiguousarray(wt_full[:, c * out_pc:(c + 1) * out_pc]),
                "wtb": (
                    np.ascontiguousarray(wtb_full[:, c * out_pc:(c + 1) * out_pc])
                    if thr_collective else wtb_full
                ),
                "biasv": np.ascontiguousarray(bias[c * out_pc:(c + 1) * out_pc]),
                "scalev": scale.reshape(1),
            }
        )
    return in_maps


def assemble_out(results, out_shape, grid_r=GRID_R, grid_c=GRID_C):
    n_tok = int(np.prod(out_shape[:-1]))
    d_out = out_shape[-1]
    tok_pc = n_tok // grid_r
    out_pc = d_out // grid_c
    full = np.empty((n_tok, d_out), dtype=np.float32)
    for cid in range(grid_r * grid_c):
        r, c = divmod(cid, grid_c)
        full[r * tok_pc:(r + 1) * tok_pc, c * out_pc:(c + 1) * out_pc] = results[cid][
            "outv"
        ]
    return full.reshape(out_shape)


def kernel(x, weight, bias, scale):
    nc = _get_full_nc()
    in_maps = make_in_maps(x, weight, bias, scale)
    res = bass_utils.run_bass_kernel_spmd(nc, in_maps, core_ids=list(range(N_CORES)))
    return assemble_out(res.results, (B, S, D_OUT))


if __name__ == "__main__":
    import reference

    inputs = reference.setup_inputs()
    out = kernel(**{k: np.asarray(v) for k, v in inputs.items()})
    print("kernel out", out.shape, out.dtype)


# revision 20
# speedup vs baseline: 1.4483x; 1.2030x over previous
"""BitLinear (ternary-weight linear + global activation requant) on 8 TRN2 cores.

Computation (see reference):
    wq  = ternarize(weight * scale, thr = 0.7*mean|weight*scale|)   # {-1,0,+1}
    out = x @ wq.T + bias
    s   = 255 / (max(out) - min(out));  out = round(out*s)/s

Sharding: 2x4 grid over (tokens, out_features).  Each core computes a
[4096 tok, 1024 out] shard contracting over the full K=4096.
x is pre-transposed/cast to bf16 on the host (layout work only); the ternary
threshold and the activation max/min are reduced across cores on-device with
two tiny AllReduces, exactly as the reference math requires.
"""

import numpy as np
import ml_dtypes

import concourse.bass as bass
import concourse.mybir as mybir
import concourse.tile as tile
from concourse import bacc
from concourse import bass_utils

F32 = mybir.dt.float32
BF16 = mybir.dt.bfloat16
F16 = mybir.dt.float16

# Full problem shape
B, S, D_IN, D_OUT = 4, 2048, 4096, 4096
N_CORES = 8
GRID_R, GRID_C = 2, 4  # token shards x out-feature shards

# Round-to-nearest-even magic constant (valid for |y| < 2^22)
RND_C = float(np.float32(12582912.0))  # 1.5 * 2^23


def build_kernel(
    tok_per_core: int,
    k_dim: int,
    out_per_core: int,
    tok_block: int,
    n_weight_copies: int,
    debug: bool = False,
    repeat: int = 1,
    rep_w: int = 1,
    rep_mm: int = 1,
    rep_tail: int = 1,
    use_collectives: bool = True,
    thr_collective: bool = True,
    stage_f16: bool = False,
    mm_no_drain: bool = False,
    mm_share_x: bool = False,
):
    """Build + compile the per-core SPMD Bass program.

    rep_* repeat individual phases in-NEFF (timing instrumentation only;
    results are unchanged since repeated phases recompute identical data).
    """
    KO = k_dim // 128
    SUBS = tok_block // 128
    OGS = max(1, out_per_core // 512)
    OGW = min(512, out_per_core)  # o-group width
    N_BLOCKS = tok_per_core // tok_block
    assert KO * 128 == k_dim and SUBS * 128 == tok_block
    assert OGS * OGW == out_per_core and N_BLOCKS * tok_block == tok_per_core

    nc = bacc.Bacc(
        "TRN2",
        target_bir_lowering=False,
        debug=debug,
        enable_asserts=False,
        num_devices=N_CORES,
    )

    xt = nc.declare_dram_parameter("xt", [N_BLOCKS, k_dim, tok_block], BF16, isOutput=False)
    wt = nc.declare_dram_parameter("wt", [k_dim, out_per_core], F32, isOutput=False)
    biasv = nc.declare_dram_parameter("biasv", [out_per_core], F32, isOutput=False)
    scalev = nc.declare_dram_parameter("scalev", [1], F32, isOutput=False)
    out = nc.declare_dram_parameter("outv", [tok_per_core, out_per_core], F32, isOutput=True)

    # raw (pre-quant) output staging in DRAM
    SDT = F16 if stage_f16 else F32
    stage = nc.dram_tensor("stage", [tok_per_core, out_per_core], SDT)

    xt_ap = xt.ap()
    wt_ap = wt.ap()
    stage_ap = stage.ap()
    out_ap = out.ap()

    n_drains = N_BLOCKS * SUBS * OGS

    with tile.TileContext(nc) as tc:
        with (
            tc.tile_pool(name="const", bufs=1) as const_pool,
            tc.tile_pool(name="wslab", bufs=4) as wslab_pool,
            tc.tile_pool(name="wq", bufs=1) as wq_pool,
            tc.tile_pool(name="xbuf", bufs=2) as x_pool,
            tc.tile_pool(name="drain", bufs=3) as drain_pool,
            tc.tile_pool(name="qt", bufs=2) as q_pool,
            tc.tile_pool(name="psum", bufs=1, space="PSUM") as psum_pool,
            tc.tile_pool(name="dram", bufs=1, space="DRAM") as dram_pool,
        ):

            def phase_consts():
                scale_sb = const_pool.tile([1, 1], F32, tag="scale_sb")
                nc.sync.dma_start(scale_sb, scalev.ap()[None, :])
                scale_b = const_pool.tile([128, 1], F32, tag="scale_b")
                nc.gpsimd.partition_broadcast(scale_b, scale_sb)

                bias_sb = const_pool.tile([1, out_per_core], F32, tag="bias_sb")
                nc.sync.dma_start(bias_sb, biasv.ap()[None, :])
                bias_b = const_pool.tile([128, out_per_core], F32, tag="bias_b")
                nc.gpsimd.partition_broadcast(bias_b, bias_sb)
                return scale_sb, scale_b, bias_b

            def phase_w(scale_sb, scale_b):
                """|W|*|scale| global mean -> threshold -> ternarize to bf16.

                The abs-sum runs on a bf16 copy of W (half the DMA, RNE
                rounding is mean-neutral); ternarize compares use fp32 W.
                """
                wsum = const_pool.tile([128, KO], F32, tag="wsum")
                for ko in range(KO):
                    wb = wslab_pool.tile(
                        [128, out_per_core], F32, tag="wbslab", bufs=3
                    )
                    nc.sync.dma_start(wb, wt_ap[ko * 128:(ko + 1) * 128, :])
                    nc.vector.tensor_reduce(
                        wsum[:, ko:ko + 1], wb,
                        axis=mybir.AxisListType.X,
                        op=mybir.AluOpType.add, apply_absolute_value=True,
                    )

                wsum1 = const_pool.tile([128, 1], F32, tag="wsum1")
                nc.vector.tensor_reduce(
                    wsum1, wsum, axis=mybir.AxisListType.X, op=mybir.AluOpType.add
                )
                wsum_all = const_pool.tile([128, 1], F32, tag="wsum_all")
                nc.gpsimd.partition_all_reduce(
                    wsum_all, wsum1, 128, bass.bass_isa.ReduceOp.add
                )

                if thr_collective:
                    cc1_in = dram_pool.tile([1, 1], F32, tag="cc1_in")
                    cc1_out = dram_pool.tile([1, 1], F32, tag="cc1_out")
                    nc.sync.dma_start(cc1_in, wsum_all[0:1, :])
                    if use_collectives:
                        nc.gpsimd.collective_compute(
                            "AllReduce",
                            mybir.AluOpType.add,
                            replica_groups=[list(range(N_CORES))],
                            ins=[cc1_in.opt()],
                            outs=[cc1_out.opt()],
                        )
                    else:
                        nc.sync.dma_start(cc1_out, cc1_in)
                    s_glob = const_pool.tile([1, 1], F32, tag="s_glob")
                    nc.sync.dma_start(s_glob, cc1_out)
                else:
                    s_glob = wsum_all[0:1, :]

                # thr2 = [t, -t];  t = 0.7 * (S_global/n_copies) / n_elems(W)
                n_w_elems = float(k_dim * GRID_C * out_per_core)
                n_cp = n_weight_copies * N_CORES // (GRID_R * GRID_C) if thr_collective else 1
                tcoef = float(np.float32(0.7) / np.float64(n_cp * n_w_elems))
                thr_c = const_pool.tile([1, 2], F32, tag="thr_c")
                nc.vector.memset(thr_c[:, 0:1], tcoef)
                nc.vector.memset(thr_c[:, 1:2], -tcoef)
                absscale = const_pool.tile([1, 1], F32, tag="absscale")
                nc.vector.tensor_reduce(
                    absscale, scale_sb, axis=mybir.AxisListType.X,
                    op=mybir.AluOpType.max, apply_absolute_value=True,
                )
                thr2 = const_pool.tile([1, 2], F32, tag="thr2")
                nc.vector.tensor_scalar_mul(thr2, thr_c, s_glob)
                nc.vector.tensor_scalar_mul(thr2, thr2, absscale)
                thr_b = const_pool.tile([128, 2], F32, tag="thr_b")
                nc.gpsimd.partition_broadcast(thr_b, thr2)

                wq = wq_pool.tile([128, KO, out_per_core], BF16, tag="wq")
                for ko in range(KO):
                    wslab = wslab_pool.tile([128, out_per_core], F32, tag="wslab")
                    nc.sync.dma_start(wslab, wt_ap[ko * 128:(ko + 1) * 128, :])
                    ws = wslab_pool.tile([128, out_per_core], F32, tag="wslab")
                    nc.vector.tensor_scalar_mul(ws, wslab, scale_b)
                    g = wslab_pool.tile([128, out_per_core], BF16, tag="tern_g", bufs=3)
                    l = wslab_pool.tile([128, out_per_core], BF16, tag="tern_l", bufs=3)
                    nc.vector.tensor_scalar(
                        g, ws, thr_b[:, 0:1], None, mybir.AluOpType.is_gt
                    )
                    nc.vector.tensor_scalar(
                        l, ws, thr_b[:, 1:2], None, mybir.AluOpType.is_lt
                    )
                    nc.vector.tensor_sub(wq[:, ko, :], g, l)
                return wq

            def phase_mm(wq, bias_b):
                """Matmul blocks: accumulate K in PSUM, +bias, max/min, stage."""
                maxst = const_pool.tile([128, n_drains], F32, tag="maxst")
                minst = const_pool.tile([128, n_drains], F32, tag="minst")
                if mm_no_drain:  # timing-only variant: stats never written
                    nc.vector.memset(maxst, 1.0)
                    nc.vector.memset(minst, -1.0)

                for blk in range(N_BLOCKS):
                    if mm_share_x and blk > 0:
                        pass  # timing-only: reuse previous x_tile
                    else:
                        x_tile = x_pool.tile([128, KO, tok_block], BF16, tag="x_tile")
                        nc.sync.dma_start(
                            x_tile, xt_ap[blk].rearrange("(ko p) t -> p ko t", p=128)
                        )
                    psums = [
                        [
                            psum_pool.tile([128, OGW], F32, name=f"ps_{sub}_{og}")
                            for og in range(OGS)
                        ]
                        for sub in range(SUBS)
                    ]
                    for ko in range(KO):
                        for sub in range(SUBS):
                            lhsT = x_tile[:, ko, sub * 128:(sub + 1) * 128]
                            for og in range(OGS):
                                nc.tensor.matmul(
                                    psums[sub][og],
                                    lhsT,
                                    wq[:, ko, og * OGW:(og + 1) * OGW],
                                    start=(ko == 0),
                                    stop=(ko == KO - 1),
                                )
                    ds = []
                    for sub in range(SUBS):
                        for og in range(OGS):
                            d = drain_pool.tile([128, OGW], SDT, tag="drain", bufs=10)
                            nc.vector.tensor_add(
                                d, psums[sub][og], bias_b[:, og * OGW:(og + 1) * OGW]
                            )
                            ds.append((sub, og, d))
                    for sub, og, d in ds:
                        tok0 = blk * tok_block + sub * 128
                        if not mm_no_drain:
                            idx = (blk * SUBS + sub) * OGS + og
                            nc.vector.tensor_reduce(
                                maxst[:, idx:idx + 1], d, axis=mybir.AxisListType.X,
                                op=mybir.AluOpType.max,
                            )
                            nc.vector.tensor_reduce(
                                minst[:, idx:idx + 1], d, axis=mybir.AxisListType.X,
                                op=mybir.AluOpType.min,
                            )
                        nc.sync.dma_start(
                            stage_ap[tok0:tok0 + 128, og * OGW:(og + 1) * OGW], d
                        )
                return maxst, minst

            def phase_tail(maxst, minst):
                """Global max/min -> s -> requantize staged output."""
                lmax = const_pool.tile([128, 1], F32, tag="lmax")
                lmin = const_pool.tile([128, 1], F32, tag="lmin")
                nc.vector.tensor_reduce(
                    lmax, maxst, axis=mybir.AxisListType.X, op=mybir.AluOpType.max
                )
                nc.vector.tensor_reduce(
                    lmin, minst, axis=mybir.AxisListType.X, op=mybir.AluOpType.min
                )
                st2 = const_pool.tile([128, 2], F32, tag="st2")
                nc.vector.tensor_copy(out=st2[:, 0:1], in_=lmax)
                nc.vector.tensor_scalar_mul(st2[:, 1:2], lmin, -1.0)
                st2r = const_pool.tile([128, 2], F32, tag="st2r")
                nc.gpsimd.partition_all_reduce(
                    st2r, st2, 128, bass.bass_isa.ReduceOp.max
                )

                cc2_in = dram_pool.tile([1, 2], F32, tag="cc2_in")
                cc2_out = dram_pool.tile([1, 2], F32, tag="cc2_out")
                nc.sync.dma_start(cc2_in, st2r[0:1, :])
                if use_collectives:
                    nc.gpsimd.collective_compute(
                        "AllReduce",
                        mybir.AluOpType.max,
                        replica_groups=[list(range(N_CORES))],
                        ins=[cc2_in.opt()],
                        outs=[cc2_out.opt()],
                    )
                else:
                    nc.sync.dma_start(cc2_out, cc2_in)
                gst = const_pool.tile([1, 2], F32, tag="gst")
                nc.sync.dma_start(gst, cc2_out)

                rng = const_pool.tile([1, 1], F32, tag="rng")  # max - min
                nc.vector.tensor_reduce(
                    rng, gst, axis=mybir.AxisListType.X, op=mybir.AluOpType.add
                )

                def accurate_recip(out_ap2, in_ap, tag):
                    # r1 = r0*(2 - x*r0), one Newton step on InstReciprocal
                    r0 = const_pool.tile([1, 1], F32, tag=f"{tag}_r0")
                    nc.vector.reciprocal(r0, in_ap)
                    e = const_pool.tile([1, 1], F32, tag=f"{tag}_e")
                    nc.vector.tensor_scalar(
                        e, in_ap, r0, None, mybir.AluOpType.mult
                    )
                    nc.vector.tensor_scalar(
                        e, e, -1.0, 2.0, mybir.AluOpType.mult, mybir.AluOpType.add
                    )
                    nc.vector.tensor_mul(out_ap2, r0, e)

                sq = const_pool.tile([1, 2], F32, tag="sq")  # [s, 1/s]
                rinv = const_pool.tile([1, 1], F32, tag="rinv")
                accurate_recip(rinv, rng, "rr")
                nc.vector.tensor_scalar_mul(sq[:, 0:1], rinv, 255.0)
                accurate_recip(sq[:, 1:2], sq[:, 0:1], "si")
                sq_b = const_pool.tile([128, 2], F32, tag="sq_b")
                nc.gpsimd.partition_broadcast(sq_b, sq)

                # q = round(y*s)/s with RNE via +/- 1.5*2^23
                CHUNK = 1  # 128-row groups per quantize tile
                n_chunks = (tok_per_core // 128) // CHUNK
                stage_r = stage_ap.rearrange("(n p) o -> p n o", p=128)
                out_r = out_ap.rearrange("(n p) o -> p n o", p=128)
                for i in range(n_chunks):
                    q = q_pool.tile([128, CHUNK, out_per_core], F32, tag="q", bufs=2)
                    if stage_f16:
                        qh = q_pool.tile(
                            [128, CHUNK, out_per_core], SDT, tag="qh", bufs=3
                        )
                        nc.sync.dma_start(qh, stage_r[:, i * CHUNK:(i + 1) * CHUNK, :])
                    else:
                        qh = q
                        nc.sync.dma_start(q, stage_r[:, i * CHUNK:(i + 1) * CHUNK, :])
                    nc.vector.tensor_scalar(
                        q, qh, sq_b[:, 0:1], RND_C,
                        mybir.AluOpType.mult, mybir.AluOpType.add,
                    )
                    nc.vector.tensor_scalar(
                        q, q, RND_C, sq_b[:, 1:2],
                        mybir.AluOpType.subtract, mybir.AluOpType.mult,
                    )
                    nc.sync.dma_start(out_r[:, i * CHUNK:(i + 1) * CHUNK, :], q)

            for _ in range(repeat):
                scale_sb, scale_b, bias_b = phase_consts()
                for _ in range(rep_w):
                    wq = phase_w(scale_sb, scale_b)
                for _ in range(rep_mm):
                    maxst, minst = phase_mm(wq, bias_b)
                for _ in range(rep_tail):
                    phase_tail(maxst, minst)

    nc.compile()
    return nc


_NC_CACHE: dict = {}


def _get_full_nc():
    key = "full"
    if key not in _NC_CACHE:
        _NC_CACHE[key] = build_kernel(
            tok_per_core=(B * S) // GRID_R,
            k_dim=D_IN,
            out_per_core=D_OUT // GRID_C,
            tok_block=512,
            n_weight_copies=GRID_R,
            debug=False,
        )
    return _NC_CACHE[key]


def make_in_maps(x, weight, bias, scale, grid_r=GRID_R, grid_c=GRID_C,
                 tok_block=512, thr_collective=True):
    """Host-side layout prep: transpose/cast/shard. No arithmetic on values."""
    x = np.asarray(x, dtype=np.float32)
    weight = np.asarray(weight, dtype=np.float32)
    bias = np.asarray(bias, dtype=np.float32)
    scale = np.asarray(scale, dtype=np.float32)

    n_tok = x.size // x.shape[-1]
    k_dim = x.shape[-1]
    d_out = weight.shape[0]
    tok_pc = n_tok // grid_r
    out_pc = d_out // grid_c
    n_blocks = tok_pc // tok_block

    xf = x.reshape(n_tok, k_dim)
    # [k, n_tok] bf16 (single transpose+cast pass)
    xtb = xf.T.astype(ml_dtypes.bfloat16)
    wt_full = np.ascontiguousarray(weight.T)  # [k, d_out]

    in_maps = []
    for cid in range(grid_r * grid_c):
        r, c = divmod(cid, grid_c)
        xs = xtb[:, r * tok_pc:(r + 1) * tok_pc]  # [k, tok_pc]
        # -> [n_blocks, k, tok_block]
        xs = np.ascontiguousarray(
            xs.reshape(k_dim, n_blocks, tok_block).transpose(1, 0, 2)
        )
        in_maps.append(
            {
                "xt": xs,
                "wt": np.ascontiguousarray(wt_full[:, c * out_pc:(c + 1) * out_pc]),
                "biasv": np.ascontiguousarray(bias[c * out_pc:(c + 1) * out_pc]),
                "scalev": scale.reshape(1),
            }
        )
    return in_maps


def assemble_out(results, out_shape, grid_r=GRID_R, grid_c=GRID_C):
    n_tok = int(np.prod(out_shape[:-1]))
    d_out = out_shape[-1]
    tok_pc = n_tok // grid_r
    out_pc = d_out // grid_c
    full = np.empty((n_tok, d_out), dtype=np.float32)
    for cid in range(grid_r * grid_c):
        r, c = divmod(cid, grid_c)
        full[r * tok_pc:(r + 1) * tok_pc, c * out_pc:(c + 1) * out_pc] = results[cid][
            "outv"
        ]
    return full.reshape(out_shape)


def kernel(x, weight, bias, scale):
    nc = _get_full_nc()
    in_maps = make_in_maps(x, weight, bias, scale)
    res = bass_utils.run_bass_kernel_spmd(nc, in_maps, core_ids=list(range(N_CORES)))
    return assemble_out(res.results, (B, S, D_OUT))


if __name__ == "__main__":
    import reference

    inputs = reference.setup_inputs()
    out = kernel(**{k: np.asarray(v) for k, v in inputs.items()})
    print("kernel out", out.shape, out.dtype)
